# revision 16
# baseline (speedup 1.0000x reference)
"""Trainium2 Bass kernel for nn_PointEncoder (PointNet-style encoder).

Data-parallel over 8 NeuronCores: 256 samples -> 32 per core.

v2 design (evacuation-bound kernel; ScalarE+VectorE are the bottleneck):
- All 64x64 matmuls (lyr/glyr/proj-tail/pool-v) use 2x2 PE-array tiling
  (tile_position): 4 concurrent 64x64 quadrant streams (measured 2.3x
  faster than 128x128 block-diag serial matmuls end-to-end).
- PSUM evacuation (relu+bias drains, 18x [128,1024] chunks/sample) split
  between ScalarE (activation) and VectorE (tensor_scalar) via DRAIN_PLAN
  (13 S / 5 V; S and V near-balanced, both ~10.5us/sample).
- Max-pools via a tensor_tensor max TREE on fp16 SBUF (2x DVE mode:
  1024-TT + 512-TT + 256-TT + 256-reduce ~ 0.8us vs 2.2us for a flat
  tensor_reduce); cross-half max via a cross-partition-base tensor_copy
  (legal for single-input DVE ops) + tensor_max instead of an SBUF DMA.
- G2 pool matvec -> dedicated PSUM bank of [128,1] slots (keeps the 3-deep
  pwork rotation free; putting it in the rotation or group-batching it
  measured much slower due to slot-holding / cross-sample coupling).
- proj-out tail: 2 psum tensor_reduce + max chain.
- Samples interleaved depth-wide so drain/pool chains of one sample hide
  behind other samples' work.
"""
import sys
import numpy as np

sys.path.insert(0, "/opt/trn_rl_repo")

import concourse.bass as bass
import concourse.bacc as bacc
import concourse.mybir as mybir
from concourse import tile
from concourse.bass_utils import run_bass_kernel_spmd

F16 = mybir.dt.float16
F32 = mybir.dt.float32
AX = mybir.AluOpType
AF = mybir.ActivationFunctionType

N_CORES = 8
B_FULL = 256
NSAMP = B_FULL // N_CORES   # 32
L = 4096
H = 64
NL = 4
LH = L // 2                 # 2048

# fp16 const columns
C16_WPI = 0            # [0:6, 0:128] block-diag proj_in
C16_WLYR = 128         # 4 x 128 block-diag (top blk parts 0:64 cols +0:64,
C16_WGLYR = 640        #                     bot blk parts 64:128 cols +64:128)
C16_WPROJ = 1152
C16_WG2 = 1664         # 4 x 64, G2.T at parts 0:64 (row-group-0 weights)
C16_WG2BD = 1920       # 4 x 128, [G2.T | G2.T] for block-diag matvec
C16_COLS = 2432

MM_MODE = "quad"       # "quad" (2x2 tile_position) or "bd" (128x128 block-diag)
# fp32 const columns (biases only)
C32_LBS = 0
C32_GBS = 4
C32_BPI = 8
C32_PBS = 9
C32_ZERO = 10
C32_COLS = 11

# drain lanes
S_DIR = 0    # scalar activation Relu(psum + bias) -> fp16
V_DIR = 1    # vector tensor_scalar (psum+bias) max 0 -> fp16

# per-sample chunk plan: h0:[a,b], lyr_i:[a,b], glyr_i:[a,b]
DRAIN_PLAN = {
    "h0": (S_DIR, S_DIR),
    "lyr": ((S_DIR, V_DIR), (S_DIR, V_DIR), (S_DIR, V_DIR), (S_DIR, V_DIR)),
    "glyr": ((S_DIR, S_DIR), (S_DIR, S_DIR), (S_DIR, S_DIR), (S_DIR, V_DIR)),
}

def build_nc(nsamp: int = NSAMP, repeat: int = 1,
             plan: dict = DRAIN_PLAN, depth: int = 5) -> bass.Bass:
    nc = bacc.Bacc()

    xT_d = nc.declare_dram_parameter("xT", [nsamp, 6, LH], F16, isOutput=False)
    c16_d = nc.declare_dram_parameter("c16", [128, C16_COLS], F16, isOutput=False)
    c32_d = nc.declare_dram_parameter("c32", [128, C32_COLS], F32, isOutput=False)
    out_d = nc.declare_dram_parameter("out", [nsamp, H], F32, isOutput=True)

    with tile.TileContext(nc) as tc:
        with (
            tc.tile_pool(name="consts", bufs=1) as cpool,
            tc.tile_pool(name="xin", bufs=depth + 1) as xpool,
            tc.tile_pool(name="acts", bufs=depth + 1) as hpool,
            tc.tile_pool(name="amid", bufs=depth + 1) as apool,
            tc.tile_pool(name="xs", bufs=4 * depth + 2) as xspool,
            tc.tile_pool(name="tiny", bufs=10 * depth) as tpool,
            tc.tile_pool(name="tree", bufs=depth + 1) as trpool,
            tc.tile_pool(name="ocoll", bufs=1) as opool,
            tc.tile_pool(name="pwork", bufs=3, space=bass.MemorySpace.PSUM) as pwork,
            tc.tile_pool(name="pvb", bufs=1, space=bass.MemorySpace.PSUM) as pvpool,
        ):
            c16 = cpool.tile([128, C16_COLS], F16, tag="c16")
            nc.sync.dma_start(c16[:], c16_d[:])
            c32 = cpool.tile([128, C32_COLS], F32, tag="c32")
            nc.sync.dma_start(c32[:], c32_d[:])

            wpi = c16[0:6, 0:128]
            # 64x64 quadrant weight blocks: top (row-group 0) / bottom (64)
            wlyr_t = lambda i: c16[0:64, C16_WLYR + 128 * i : C16_WLYR + 128 * i + 64]
            wlyr_b = lambda i: c16[64:128, C16_WLYR + 128 * i + 64 : C16_WLYR + 128 * i + 128]
            wglyr_t = lambda i: c16[0:64, C16_WGLYR + 128 * i : C16_WGLYR + 128 * i + 64]
            wglyr_b = lambda i: c16[64:128, C16_WGLYR + 128 * i + 64 : C16_WGLYR + 128 * i + 128]
            wproj_t = lambda i: c16[0:64, C16_WPROJ + 128 * i : C16_WPROJ + 128 * i + 64]
            wproj_b = lambda i: c16[64:128, C16_WPROJ + 128 * i + 64 : C16_WPROJ + 128 * i + 128]
            wg2 = lambda i: c16[0:64, C16_WG2 + 64 * i : C16_WG2 + 64 * i + 64]
            wg2bd = lambda i: c16[0:64, C16_WG2BD + 128 * i : C16_WG2BD + 128 * i + 128]
            wlyr_f = lambda i: c16[:, C16_WLYR + 128 * i : C16_WLYR + 128 * (i + 1)]
            wglyr_f = lambda i: c16[:, C16_WGLYR + 128 * i : C16_WGLYR + 128 * (i + 1)]
            wproj_f = lambda i: c16[:, C16_WPROJ + 128 * i : C16_WPROJ + 128 * (i + 1)]
            lbs = lambda i: c32[:, C32_LBS + i : C32_LBS + i + 1]
            gbs = lambda i: c32[:, C32_GBS + i : C32_GBS + i + 1]
            bpi = c32[:, C32_BPI : C32_BPI + 1]
            pbs_top = c32[0:64, C32_PBS : C32_PBS + 1]

            outcoll = opool.tile([64, nsamp], F32, tag="outc")
            # one PSUM bank of [128,1] slots for the G2 matvec results
            pvb = pvpool.tile([128, 512], F32, tag="pvb")
            pv_ctr = [0]

            def mm4(ps, w_t, w_b, src, c0):
                """Four concurrent 64x64 quadrant matmuls: input cols
                [c0:c0+1024] of src -> ps[:, 0:1024]."""
                a, b = c0, c0 + 512
                nc.tensor.matmul(ps[0:64, 0:512], w_t, src[0:64, a:a + 512],
                                 start=True, stop=True, tile_position=(0, 0))
                nc.tensor.matmul(ps[64:128, 0:512], w_t, src[0:64, b:b + 512],
                                 start=True, stop=True, tile_position=(0, 64))
                nc.tensor.matmul(ps[0:64, 512:1024], w_b, src[64:128, a:a + 512],
                                 start=True, stop=True, tile_position=(64, 0))
                nc.tensor.matmul(ps[64:128, 512:1024], w_b, src[64:128, b:b + 512],
                                 start=True, stop=True, tile_position=(64, 64))

            def mmbd(ps, w_f, src, c0):
                """Two serial full-width block-diag matmuls: input cols
                [c0:c0+1024] of src -> ps[:, 0:1024]."""
                nc.tensor.matmul(ps[:, 0:512], w_f, src[:, c0:c0 + 512],
                                 start=True, stop=True)
                nc.tensor.matmul(ps[:, 512:1024], w_f, src[:, c0 + 512:c0 + 1024],
                                 start=True, stop=True)

            def drain(lane, dst, ps, bias):
                """relu(ps + bias) -> dst (fp16 [128,1024] slice)."""
                if lane == S_DIR:
                    nc.scalar.activation(dst, ps[:], AF.Relu, bias=bias, scale=1.0)
                else:
                    nc.vector.tensor_scalar(out=dst, in0=ps[:], scalar1=bias,
                                            scalar2=0.0, op0=AX.add, op1=AX.max)

            def st_load(st):
                st["xt"] = xpool.tile([6, LH], F16, tag="xt", name=f"xt_{st['s']}")
                nc.sync.dma_start(st["xt"][:], xT_d[st["s"]])

            def st_projin(st):
                h1 = hpool.tile([128, LH], F16, tag="h1")
                pl = plan["h0"]
                for t in range(2):
                    ps = pwork.tile([128, 1024], F32, tag="pw")
                    for c in range(2):
                        o = 1024 * t + 512 * c
                        nc.tensor.matmul(
                            ps[:, 512 * c : 512 * c + 512], wpi,
                            st["xt"][:, o : o + 512], start=True, stop=True,
                        )
                    drain(pl[t], h1[:, 1024 * t : 1024 * t + 1024], ps, bpi)
                st["cur"] = h1
                st["xs"] = []

            def st_lyr(st, i):
                at = apool.tile([128, LH], F16, tag="at")
                pl = plan["lyr"][i]
                for t in range(2):
                    ps = pwork.tile([128, 1024], F32, tag="pw")
                    if MM_MODE == "bd":
                        mmbd(ps, wlyr_f(i), st["cur"], 1024 * t)
                    else:
                        mm4(ps, wlyr_t(i), wlyr_b(i), st["cur"], 1024 * t)
                    drain(pl[t], at[:, 1024 * t : 1024 * t + 1024], ps, lbs(i))
                st["at"] = at

            def st_pool_a1(st, i):
                # pool via TT-max tree over at (fp16 SBUF, 2x mode)
                t1 = trpool.tile([128, 1024], F16, tag="t1")
                at = st["at"]
                nc.vector.tensor_max(t1[:], at[:, 0:1024], at[:, 1024:2048])
                st["t1"] = t1

            def st_pool_a2(st, i):
                t2 = trpool.tile([128, 512], F16, tag="t2")
                t1 = st["t1"]
                nc.vector.tensor_max(t2[:], t1[:, 0:512], t1[:, 512:1024])
                st["t2"] = t2

            def st_pool_a3(st, i):
                t3 = trpool.tile([128, 256], F16, tag="t3")
                t2 = st["t2"]
                nc.vector.tensor_max(t3[:], t2[:, 0:256], t2[:, 256:512])
                st["t3"] = t3

            def st_pool_a4(st, i):
                m = tpool.tile([128, 1], F32, tag="m")
                nc.vector.tensor_reduce(out=m[:, 0:1], in_=st["t3"][:],
                                        axis=mybir.AxisListType.X, op=AX.max)
                st["m"] = m

            def st_pool_b(st, i):
                # cross-half: gx[c] = max(m[c], m[c+64]) via cross-base copy
                m = st["m"]
                sh = tpool.tile([64, 1], F32, tag="sh")
                nc.vector.tensor_copy(sh[:, 0:1], m[64:128, 0:1])
                gx = tpool.tile([64, 1], F16, tag="gx")
                nc.vector.tensor_max(gx[:], m[0:64, 0:1], sh[:])
                st["gx"] = gx

            def st_pool_c(st, i):
                # v_pre = G2 @ gx  into a [128,1] psum slot (both halves)
                j = pv_ctr[0] % 512
                pv_ctr[0] += 1
                slot = pvb[:, j : j + 1]
                nc.tensor.matmul(pvb[0:64, j : j + 1], wg2(i), st["gx"][:],
                                 start=True, stop=True, tile_position=(0, 0))
                nc.tensor.matmul(pvb[64:128, j : j + 1], wg2(i), st["gx"][:],
                                 start=True, stop=True, tile_position=(0, 64))
                st["pv"] = slot

            def st_pool_d(st, i):
                v = tpool.tile([128, 1], F32, tag="v")
                nc.vector.tensor_scalar_add(v[:], st["pv"], gbs(i))
                st["v"] = v

            def st_glyr(st, i):
                xs_i = xspool.tile([128, LH], F16, tag="xs")
                pl = plan["glyr"][i]
                for t in range(2):
                    ps = pwork.tile([128, 1024], F32, tag="pw")
                    if MM_MODE == "bd":
                        mmbd(ps, wglyr_f(i), st["at"], 1024 * t)
                    else:
                        mm4(ps, wglyr_t(i), wglyr_b(i), st["at"], 1024 * t)
                    drain(pl[t], xs_i[:, 1024 * t : 1024 * t + 1024], ps,
                          st["v"][:, 0:1])
                st["xs"].append(xs_i)
                st["cur"] = xs_i

            def st_tail_t(st, t):
                # proj-out psum chunk t, then fused half-pair max-reduce:
                # macc[c] = max(prev, max_cols(max(pt[c, :], pt[c+64, :])))
                pt = pwork.tile([128, 1024], F32, tag="pw")
                a, b = 1024 * t, 1024 * t + 512
                if MM_MODE == "bd":
                    for i in range(NL):
                        s0, s1 = (i == 0), (i == NL - 1)
                        nc.tensor.matmul(pt[:, 0:512], wproj_f(i),
                                         st["xs"][i][:, a:a + 512],
                                         start=s0, stop=s1)
                    for i in range(NL):
                        s0, s1 = (i == 0), (i == NL - 1)
                        nc.tensor.matmul(pt[:, 512:1024], wproj_f(i),
                                         st["xs"][i][:, b:b + 512],
                                         start=s0, stop=s1)
                else:
                    for i in range(NL):
                        s0, s1 = (i == 0), (i == NL - 1)
                        nc.tensor.matmul(pt[0:64, 0:512], wproj_t(i),
                                         st["xs"][i][0:64, a:a + 512],
                                         start=s0, stop=s1, tile_position=(0, 0))
                    for i in range(NL):
                        s0, s1 = (i == 0), (i == NL - 1)
                        nc.tensor.matmul(pt[64:128, 0:512], wproj_t(i),
                                         st["xs"][i][0:64, b:b + 512],
                                         start=s0, stop=s1, tile_position=(0, 64))
                    for i in range(NL):
                        s0, s1 = (i == 0), (i == NL - 1)
                        nc.tensor.matmul(pt[0:64, 512:1024], wproj_b(i),
                                         st["xs"][i][64:128, a:a + 512],
                                         start=s0, stop=s1, tile_position=(64, 0))
                    for i in range(NL):
                        s0, s1 = (i == 0), (i == NL - 1)
                        nc.tensor.matmul(pt[64:128, 512:1024], wproj_b(i),
                                         st["xs"][i][64:128, b:b + 512],
                                         start=s0, stop=s1, tile_position=(64, 64))
                macc = tpool.tile([128, 1], F32, tag=f"macc{t}")
                nc.vector.tensor_reduce(out=macc[:, 0:1], in_=pt[:],
                                        axis=mybir.AxisListType.X, op=AX.max)
                st[f"macc{t}"] = macc

            def st_tail_fin(st):
                mp = tpool.tile([128, 1], F32, tag="mp")
                nc.vector.tensor_max(mp[:], st["macc0"][:], st["macc1"][:])
                shp = tpool.tile([64, 1], F32, tag="shp")
                nc.vector.tensor_copy(shp[:, 0:1], mp[64:128, 0:1])
                fin = tpool.tile([64, 1], F32, tag="fin")
                nc.vector.tensor_max(fin[:], mp[0:64, 0:1], shp[:])
                nc.vector.tensor_scalar_add(
                    outcoll[:, st["s"] : st["s"] + 1], fin[:], pbs_top
                )

            order = [s for _ in range(repeat) for s in range(nsamp)]
            for g0 in range(0, len(order), depth):
                grp = [{"s": s} for s in order[g0 : g0 + depth]]
                for st in grp:
                    st_load(st)
                for st in grp:
                    st_projin(st)
                for i in range(NL):
                    for st in grp:
                        st_lyr(st, i)
                    for st in grp:
                        st_pool_a1(st, i)
                    for st in grp:
                        st_pool_a2(st, i)
                    for st in grp:
                        st_pool_a3(st, i)
                    for st in grp:
                        st_pool_a4(st, i)
                    for st in grp:
                        st_pool_b(st, i)
                    for st in grp:
                        st_pool_c(st, i)
                    for st in grp:
                        st_pool_d(st, i)
                    for st in grp:
                        st_glyr(st, i)
                for t in range(2):
                    for st in grp:
                        st_tail_t(st, t)
                for st in grp:
                    st_tail_fin(st)

            nc.sync.dma_start(out_d[:].rearrange("s e -> e s"), outcoll[:])

    nc.finalize()
    return nc


def prep_maps(x: np.ndarray, proj_in_w, proj_in_b, lyr_w, lyr_b, glyr_w,
              glyr_b, proj_out_w, proj_out_b, nsamp: int = NSAMP,
              n_cores: int = N_CORES):
    B = x.shape[0]
    xT = np.ascontiguousarray(
        x.reshape(B, 2, LH, 3).transpose(0, 1, 3, 2)
    ).reshape(B, 6, LH).astype(np.float16)

    def diag2(w):
        z = np.zeros((128, 128), np.float32)
        z[0:64, 0:64] = w.T
        z[64:128, 64:128] = w.T
        return z

    G1 = glyr_w[:, :, :H]
    G2 = glyr_w[:, :, H:]
    P = proj_out_w.reshape(H, NL, H).transpose(1, 0, 2)

    c16 = np.zeros((128, C16_COLS), np.float32)
    c16[0:3, 0:64] = proj_in_w.T
    c16[3:6, 64:128] = proj_in_w.T
    for i in range(NL):
        c16[:, C16_WLYR + 128 * i : C16_WLYR + 128 * (i + 1)] = diag2(lyr_w[i])
        c16[:, C16_WGLYR + 128 * i : C16_WGLYR + 128 * (i + 1)] = diag2(G1[i])
        c16[:, C16_WPROJ + 128 * i : C16_WPROJ + 128 * (i + 1)] = diag2(P[i])
        c16[0:64, C16_WG2 + 64 * i : C16_WG2 + 64 * (i + 1)] = G2[i].T
        c16[0:64, C16_WG2BD + 128 * i : C16_WG2BD + 128 * i + 64] = G2[i].T
        c16[0:64, C16_WG2BD + 128 * i + 64 : C16_WG2BD + 128 * (i + 1)] = G2[i].T

    c32 = np.zeros((128, C32_COLS), np.float32)
    for i in range(NL):
        c32[:, C32_LBS + i] = np.tile(lyr_b[i], 2)
        c32[:, C32_GBS + i] = np.tile(glyr_b[i], 2)
    c32[:, C32_BPI] = np.tile(proj_in_b, 2)
    c32[:, C32_PBS] = np.tile(proj_out_b, 2)

    const_map = {
        "c16": c16.astype(np.float16),
        "c32": c32.astype(np.float32),
    }
    in_maps = []
    for ci in range(n_cores):
        m = dict(const_map)
        m["xT"] = np.ascontiguousarray(xT[ci * nsamp : (ci + 1) * nsamp])
        in_maps.append(m)
    return in_maps


_NC_CACHE = {}


def _get_nc(nsamp=NSAMP):
    if nsamp not in _NC_CACHE:
        _NC_CACHE[nsamp] = build_nc(nsamp)
    return _NC_CACHE[nsamp]


def kernel(x, proj_in_w, proj_in_b, lyr_w, lyr_b, glyr_w, glyr_b,
           proj_out_w, proj_out_b, _trace: bool = False):
    args = [np.asarray(a) for a in
            (x, proj_in_w, proj_in_b, lyr_w, lyr_b, glyr_w, glyr_b,
             proj_out_w, proj_out_b)]
    in_maps = prep_maps(*args)
    nc = _get_nc()
    res = run_bass_kernel_spmd(nc, in_maps, list(range(N_CORES)), trace=_trace)
    out = np.concatenate([r["out"] for r in res.results], 0).astype(np.float32)
    if _trace:
        return out, res
    return out
